# revision 20
# baseline (speedup 1.0000x reference)
"""Trainium2 Bass kernel for ChebConv with spatial attention.

Reference computation (per sample b):
    A_k = cheb[k] * att[b]                    (elementwise, [N,N])
    rhs_k = A_k @ x[b,t]                      ([N,N] @ [N,F_IN] for all t)
    out[b,t] = relu(sum_k rhs_k @ Theta[k])   ([N,F_OUT])

Sharding: data-parallel over batch B=8, one sample per NeuronCore.
cheb/Theta replicated. Host prep passes transposed adjacency factors
(attT/chebT, layout [j,i], cast to bf16) so the on-chip elementwise
product directly yields A^T tiles, which the PE consumes as the moving
matmul operand with contraction over j on the partition dim — no
on-chip transposes. All matmuls run in bf16 (single-pass on the PE,
fp32 PSUM accumulation); the output is computed and stored in fp32.

Per-core dataflow:
  phase B: for k, j-pair: AT = attT*chebT (DVE bf16), then accumulate
           RT[(t,f)=128, i=512] = X_tile^T @ AT into PSUM over j-tiles
           (N=512 bf16 matmuls), copy+cast PSUM->SBUF bf16 (DVE + ACT).
  phase C: out[i=128, (t,o)] += RT^T @ thetaM_k, where thetaM zero-pads
           Theta[k] per 32-row strip so one full-K matmul produces the
           4 t's of a t-group (N=256). relu on ACT, contiguous DMA out.

DMA layout: att/cheb/x loaded as wide tiles covering two 128-row
j-tiles per transfer (rearranged so each partition carries both rows),
halving DMA count; x/att/theta ride the scalar-engine HWDGE queue while
cheb/out use the sync queue so issue overhead is split across queues.
"""

import numpy as np
from contextlib import ExitStack

B, T, N, F_IN, F_OUT, K = 8, 16, 1024, 32, 64, 3
NJ = N // 128  # j tiles (contraction)
NI = N // 128  # i tiles (output rows)
NTG = 4        # t-groups of 4 t's -> 128 = 4*32 partitions
TF = T * F_IN   # 512
TO = T * F_OUT  # 1024
W = 2           # j-tiles per wide DMA
NW = NJ // W

_LAST_RESULTS = None  # BassKernelResults of the most recent run (for test harness)


def _build_bass():
    import concourse.mybir as mybir
    import concourse.tile as tile
    from concourse import bacc
    from concourse.bass import ts

    f32 = mybir.dt.float32
    bf16 = mybir.dt.bfloat16
    nc = bacc.Bacc()

    xT_d = nc.dram_tensor("xT", [N, TF], bf16, kind="ExternalInput")
    attT_d = nc.dram_tensor("attT", [N, N], bf16, kind="ExternalInput")
    chebT_d = nc.dram_tensor("chebT", [K * N, N], bf16, kind="ExternalInput")
    th_d = nc.dram_tensor("thetaM", [128, K * 4 * F_OUT], bf16, kind="ExternalInput")
    out_d = nc.dram_tensor("out", [N, TO], bf16, kind="ExternalOutput")

    # j-tiles grouped per DMA: two narrow leading groups let the first
    # matmul start as soon as ~0.5 MB has landed; the rest go wide
    GROUPS = [(0,), (1,), (2, 3), (4, 5), (6, 7)]

    def grouped(dram, row0, L):  # L j-tiles -> [128, L, cols]
        return dram[row0:row0 + L * 128, :].rearrange("(a p) n -> p a n", p=128)

    def g3(t, L):  # view a grouped SBUF tile as [128, L, cols]
        return t[:].rearrange("p (a n) -> p a n", a=L)

    with tile.TileContext(nc) as tc, ExitStack() as ctx:
        x_pool = ctx.enter_context(tc.tile_pool(name="x", bufs=1))
        att_pool = ctx.enter_context(tc.tile_pool(name="att", bufs=1))
        cheb_pool = ctx.enter_context(tc.tile_pool(name="cheb", bufs=5))
        at_pool = ctx.enter_context(tc.tile_pool(name="at", bufs=5))
        rt_pool = ctx.enter_context(tc.tile_pool(name="rt", bufs=K * NTG))
        th_pool = ctx.enter_context(tc.tile_pool(name="th", bufs=1))
        ob_pool = ctx.enter_context(tc.tile_pool(name="ob", bufs=3))

        xg, attg = [None] * len(GROUPS), [None] * len(GROUPS)

        # phase B: RT[k][tg] = X[:, tg-block]^T @ (attT * chebT_k)
        rts = [[None] * NTG for _ in range(K)]
        th = None
        with tc.tile_pool(name="psumB", bufs=1, space="PSUM") as pb:
            # PE warm-up: ~5us of zero matmuls during the DMA head so HAM
            # un-throttles (1.2 -> 2.4 GHz) before the first real matmul.
            # Shares the last chain's PSUM slot; released before phase B
            # reaches it.
            wz = at_pool.tile([128, 512], bf16, name="warmz", tag="at")
            nc.gpsimd.memset(wz[:], 0)
            wps = pb.tile([128, 512], f32, name="warmps", tag="chain7")
            for _ in range(14):
                nc.tensor.matmul(wps[:], wz[:, 0:128], wz[:], start=True, stop=True)
            for k in range(K):
                chains = [
                    pb.tile([128, 512], f32, name=f"chain{k}_{c}", tag=f"chain{c}")
                    for c in range(2 * NTG)
                ]
                for g, grp in enumerate(GROUPS):
                    L = len(grp)
                    row0 = grp[0] * 128
                    if k == 0:
                        a = att_pool.tile([128, L * N], bf16,
                                          name=f"attg{g}", tag=f"attg{g}")
                        nc.scalar.dma_start(g3(a, L), grouped(attT_d, row0, L))
                        attg[g] = a
                        xt = x_pool.tile([128, L * TF], bf16,
                                         name=f"xg{g}", tag=f"xg{g}")
                        nc.scalar.dma_start(g3(xt, L), grouped(xT_d, row0, L))
                        xg[g] = xt
                    cb = cheb_pool.tile([128, L * N], bf16, name=f"cb{k}_{g}",
                                        tag="cb", padded_shape=[128, W * N])
                    nc.sync.dma_start(g3(cb, L), grouped(chebT_d, k * N + row0, L))
                    if k == 0 and g == len(GROUPS) - 1:
                        th = th_pool.tile([128, K * 4 * F_OUT], bf16)
                        nc.scalar.dma_start(th[:], th_d[:, :])
                    at = at_pool.tile([128, L * N], bf16, name=f"at{k}_{g}",
                                      tag="at", padded_shape=[128, W * N])
                    nc.vector.tensor_mul(at[:], attg[g][:], cb[:])
                    for js in range(L):
                        first = g == 0
                        last = (g == len(GROUPS) - 1 and js == L - 1)
                        for tg in range(NTG):
                            for ih in range(2):
                                nc.tensor.matmul(
                                    chains[tg * 2 + ih][:],
                                    xg[g][:, ts(js * NTG + tg, 128)],
                                    at[:, ts(js * 2 + ih, 512)],
                                    start=first,
                                    stop=last,
                                )
                for tg in range(NTG):
                    rt = rt_pool.tile([128, N], bf16)
                    nc.vector.tensor_copy(rt[:, 0:512], chains[tg * 2][:])
                    nc.scalar.copy(rt[:, 512:1024], chains[tg * 2 + 1][:])
                    rts[k][tg] = rt

        # phase C: out[i-block, (t,o)] = relu(sum_k RT_k^T @ thetaM_k).
        # One matmul per (tg, k): full K=128 contraction where thetaM
        # zero-pads Theta[k] per 32-row strip, producing the 4 t's of
        # the t-group in one N=256 matmul. Two single-bank PSUM tiles
        # per i-block; tg order alternates banks so only one
        # accumulation group is open per bank.
        with tc.tile_pool(name="psumC", bufs=1, space="PSUM") as pc:
            for ib in range(NI):
                psA = pc.tile([128, 512], f32, name=f"psA{ib}", tag="psA", bufs=2)
                psB = pc.tile([128, 512], f32, name=f"psB{ib}", tag="psB", bufs=2)
                for tg, ps in ((0, psA), (2, psB), (1, psA), (3, psB)):
                    for k in range(K):
                        nc.tensor.matmul(
                            ps[:, ts(tg % 2, 4 * F_OUT)],
                            rts[k][tg][:, ts(ib, 128)],
                            th[:, ts(k, 4 * F_OUT)],
                            start=(k == 0),
                            stop=(k == K - 1),
                        )
                ob = ob_pool.tile([128, TO], bf16)
                nc.scalar.activation(ob[:, 0:512], psA[:],
                                     mybir.ActivationFunctionType.Relu)
                nc.scalar.activation(ob[:, 512:1024], psB[:],
                                     mybir.ActivationFunctionType.Relu)
                nc.sync.dma_start(out_d[ts(ib, 128), :], ob[:])

    nc.compile()
    return nc


def _prep_inputs(x, att, cheb, Theta):
    import ml_dtypes

    bf16 = ml_dtypes.bfloat16
    chebT = np.ascontiguousarray(cheb.transpose(0, 2, 1)).reshape(K * N, N)
    chebT = chebT.astype(bf16)
    # zero-padded Theta: strip tt of the partition dim carries Theta[k]
    # only in the tt-th 64-col block of k's 256-col group
    thetaM = np.zeros((128, K * 4 * F_OUT), np.float32)
    for tt in range(4):
        for k in range(K):
            thetaM[tt * 32:(tt + 1) * 32,
                   k * 4 * F_OUT + tt * F_OUT:
                   k * 4 * F_OUT + (tt + 1) * F_OUT] = Theta[k]
    thetaM = thetaM.astype(bf16)

    in_maps = []
    for b in range(B):
        in_maps.append({
            "xT": np.ascontiguousarray(
                x[b].transpose(1, 0, 2)).reshape(N, TF).astype(bf16),
            "attT": np.ascontiguousarray(att[b].T).astype(bf16),
            "chebT": chebT,
            "thetaM": thetaM,
        })
    return in_maps


def kernel(**inputs: np.ndarray) -> np.ndarray:
    global _LAST_RESULTS
    from concourse.bass_utils import run_bass_kernel_spmd

    x = np.asarray(inputs["x"], dtype=np.float32)
    att = np.asarray(inputs["spatial_attention"], dtype=np.float32)
    cheb = np.asarray(inputs["cheb"], dtype=np.float32)
    Theta = np.asarray(inputs["Theta"], dtype=np.float32)

    in_maps = _prep_inputs(x, att, cheb, Theta)
    nc = _build_bass()
    res = run_bass_kernel_spmd(nc, in_maps, core_ids=list(range(B)))
    _LAST_RESULTS = res

    out = np.stack(
        [r["out"].astype(np.float32).reshape(N, T, F_OUT).transpose(1, 0, 2)
         for r in res.results]
    )
    return out


# revision 21
# speedup vs baseline: 1.1002x; 1.1002x over previous
"""Trainium2 Bass kernel for ChebConv with spatial attention.

Reference computation (per sample b):
    A_k = cheb[k] * att[b]                    (elementwise, [N,N])
    rhs_k = A_k @ x[b,t]                      ([N,N] @ [N,F_IN] for all t)
    out[b,t] = relu(sum_k rhs_k @ Theta[k])   ([N,F_OUT])

Sharding: data-parallel over batch B=8, one sample per NeuronCore.
cheb/Theta replicated. Host prep passes transposed adjacency factors
(attT/chebT, layout [j,i], cast to bf16) so the on-chip elementwise
product directly yields A^T tiles, which the PE consumes as the moving
matmul operand with contraction over j on the partition dim — no
on-chip transposes. All matmuls run in bf16 (single-pass on the PE,
fp32 PSUM accumulation); the output is computed and stored in fp32.

Per-core dataflow:
  phase B: for k, j-pair: AT = attT*chebT (DVE bf16), then accumulate
           RT[(t,f)=128, i=512] = X_tile^T @ AT into PSUM over j-tiles
           (N=512 bf16 matmuls), copy+cast PSUM->SBUF bf16 (DVE + ACT).
  phase C: out[i=128, (t,o)] += RT^T @ thetaM_k, where thetaM zero-pads
           Theta[k] per 32-row strip so one full-K matmul produces the
           4 t's of a t-group (N=256). relu on ACT, contiguous DMA out.

DMA layout: att/cheb/x loaded as wide tiles covering two 128-row
j-tiles per transfer (rearranged so each partition carries both rows),
halving DMA count; x/att/theta ride the scalar-engine HWDGE queue while
cheb/out use the sync queue so issue overhead is split across queues.
"""

import numpy as np
from contextlib import ExitStack

B, T, N, F_IN, F_OUT, K = 8, 16, 1024, 32, 64, 3
NJ = N // 128  # j tiles (contraction)
NI = N // 128  # i tiles (output rows)
NTG = 4        # t-groups of 4 t's -> 128 = 4*32 partitions
TF = T * F_IN   # 512
TO = T * F_OUT  # 1024
W = 2           # j-tiles per wide DMA
NW = NJ // W

_LAST_RESULTS = None  # BassKernelResults of the most recent run (for test harness)


def _build_bass():
    import concourse.mybir as mybir
    import concourse.tile as tile
    from concourse import bacc
    from concourse.bass import ts

    f32 = mybir.dt.float32
    bf16 = mybir.dt.bfloat16
    nc = bacc.Bacc()

    xT_d = nc.dram_tensor("xT", [N, TF], bf16, kind="ExternalInput")
    attT_d = nc.dram_tensor("attT", [N, N], bf16, kind="ExternalInput")
    chebT_d = nc.dram_tensor("chebT", [K * N, N], bf16, kind="ExternalInput")
    th_d = nc.dram_tensor("thetaM", [128, K * 4 * F_OUT], bf16, kind="ExternalInput")
    out_d = nc.dram_tensor("out", [N, TO], bf16, kind="ExternalOutput")

    # j-tiles grouped per DMA: two narrow leading groups let the first
    # matmul start as soon as ~0.5 MB has landed; the rest go wide
    GROUPS = [(0,), (1,), (2, 3), (4, 5), (6, 7)]

    def grouped(dram, row0, L):  # L j-tiles -> [128, L, cols]
        return dram[row0:row0 + L * 128, :].rearrange("(a p) n -> p a n", p=128)

    def g3(t, L):  # view a grouped SBUF tile as [128, L, cols]
        return t[:].rearrange("p (a n) -> p a n", a=L)

    with tile.TileContext(nc) as tc, ExitStack() as ctx:
        x_pool = ctx.enter_context(tc.tile_pool(name="x", bufs=1))
        att_pool = ctx.enter_context(tc.tile_pool(name="att", bufs=1))
        cheb_pool = ctx.enter_context(tc.tile_pool(name="cheb", bufs=3))
        at_pool = ctx.enter_context(tc.tile_pool(name="at", bufs=3))
        rt_pool = ctx.enter_context(tc.tile_pool(name="rt", bufs=K * NTG))
        th_pool = ctx.enter_context(tc.tile_pool(name="th", bufs=1))
        ob_pool = ctx.enter_context(tc.tile_pool(name="ob", bufs=3))

        xg, attg = [None] * len(GROUPS), [None] * len(GROUPS)

        # phase B: RT[k][tg] = X[:, tg-block]^T @ (attT * chebT_k)
        rts = [[None] * NTG for _ in range(K)]
        th = None
        with tc.tile_pool(name="psumB", bufs=1, space="PSUM") as pb:
            # PE warm-up: ~5us of zero matmuls during the DMA head so HAM
            # un-throttles (1.2 -> 2.4 GHz) before the first real matmul.
            # Shares the last chain's PSUM slot; released before phase B
            # reaches it.
            wz = at_pool.tile([128, 512], bf16, name="warmz", tag="at")
            nc.gpsimd.memset(wz[:], 0)
            wps = pb.tile([128, 512], f32, name="warmps", tag="chain7")
            for _ in range(14):
                nc.tensor.matmul(wps[:], wz[:, 0:128], wz[:], start=True, stop=True)
            for k in range(K):
                chains = [
                    pb.tile([128, 512], f32, name=f"chain{k}_{c}", tag=f"chain{c}")
                    for c in range(2 * NTG)
                ]
                for g, grp in enumerate(GROUPS):
                    L = len(grp)
                    row0 = grp[0] * 128
                    if k == 0:
                        a = att_pool.tile([128, L * N], bf16,
                                          name=f"attg{g}", tag=f"attg{g}")
                        nc.scalar.dma_start(g3(a, L), grouped(attT_d, row0, L))
                        attg[g] = a
                        xt = x_pool.tile([128, L * TF], bf16,
                                         name=f"xg{g}", tag=f"xg{g}")
                        nc.scalar.dma_start(g3(xt, L), grouped(xT_d, row0, L))
                        xg[g] = xt
                    cb = cheb_pool.tile([128, L * N], bf16, name=f"cb{k}_{g}",
                                        tag="cb", padded_shape=[128, W * N])
                    nc.sync.dma_start(g3(cb, L), grouped(chebT_d, k * N + row0, L))
                    if k == 0 and g == 1:
                        th = th_pool.tile([128, K * 4 * F_OUT], bf16)
                        nc.scalar.dma_start(th[:], th_d[:, :])
                    at = at_pool.tile([128, L * N], bf16, name=f"at{k}_{g}",
                                      tag="at", padded_shape=[128, W * N])
                    nc.vector.tensor_mul(at[:], attg[g][:], cb[:])
                    for js in range(L):
                        first = g == 0
                        last = (g == len(GROUPS) - 1 and js == L - 1)
                        for tg in range(NTG):
                            for ih in range(2):
                                nc.tensor.matmul(
                                    chains[tg * 2 + ih][:],
                                    xg[g][:, ts(js * NTG + tg, 128)],
                                    at[:, ts(js * 2 + ih, 512)],
                                    start=first,
                                    stop=last,
                                )
                for tg in range(NTG):
                    rt = rt_pool.tile([128, N], bf16)
                    nc.vector.tensor_copy(rt[:, 0:512], chains[tg * 2][:])
                    nc.scalar.copy(rt[:, 512:1024], chains[tg * 2 + 1][:])
                    rts[k][tg] = rt

        # phase C: out[i-block, (t,o)] = relu(sum_k RT_k^T @ thetaM_k).
        # One matmul per (tg, k): full K=128 contraction where thetaM
        # zero-pads Theta[k] per 32-row strip, producing the 4 t's of
        # the t-group in one N=256 matmul. Two single-bank PSUM tiles
        # per i-block; tg order alternates banks so only one
        # accumulation group is open per bank.
        with tc.tile_pool(name="psumC", bufs=1, space="PSUM") as pc:
            for ib in range(NI):
                psA = pc.tile([128, 512], f32, name=f"psA{ib}", tag="psA", bufs=2)
                psB = pc.tile([128, 512], f32, name=f"psB{ib}", tag="psB", bufs=2)
                for tg, ps in ((0, psA), (2, psB), (1, psA), (3, psB)):
                    for k in range(K):
                        nc.tensor.matmul(
                            ps[:, ts(tg % 2, 4 * F_OUT)],
                            rts[k][tg][:, ts(ib, 128)],
                            th[:, ts(k, 4 * F_OUT)],
                            start=(k == 0),
                            stop=(k == K - 1),
                        )
                ob = ob_pool.tile([128, TO], bf16)
                nc.scalar.activation(ob[:, 0:512], psA[:],
                                     mybir.ActivationFunctionType.Relu)
                nc.scalar.activation(ob[:, 512:1024], psB[:],
                                     mybir.ActivationFunctionType.Relu)
                nc.sync.dma_start(out_d[ts(ib, 128), :], ob[:])

    nc.compile()
    return nc


def _prep_inputs(x, att, cheb, Theta):
    import ml_dtypes

    bf16 = ml_dtypes.bfloat16
    chebT = np.ascontiguousarray(cheb.transpose(0, 2, 1)).reshape(K * N, N)
    chebT = chebT.astype(bf16)
    # zero-padded Theta: strip tt of the partition dim carries Theta[k]
    # only in the tt-th 64-col block of k's 256-col group
    thetaM = np.zeros((128, K * 4 * F_OUT), np.float32)
    for tt in range(4):
        for k in range(K):
            thetaM[tt * 32:(tt + 1) * 32,
                   k * 4 * F_OUT + tt * F_OUT:
                   k * 4 * F_OUT + (tt + 1) * F_OUT] = Theta[k]
    thetaM = thetaM.astype(bf16)

    in_maps = []
    for b in range(B):
        in_maps.append({
            "xT": np.ascontiguousarray(
                x[b].transpose(1, 0, 2)).reshape(N, TF).astype(bf16),
            "attT": np.ascontiguousarray(att[b].T).astype(bf16),
            "chebT": chebT,
            "thetaM": thetaM,
        })
    return in_maps


def kernel(**inputs: np.ndarray) -> np.ndarray:
    global _LAST_RESULTS
    from concourse.bass_utils import run_bass_kernel_spmd

    x = np.asarray(inputs["x"], dtype=np.float32)
    att = np.asarray(inputs["spatial_attention"], dtype=np.float32)
    cheb = np.asarray(inputs["cheb"], dtype=np.float32)
    Theta = np.asarray(inputs["Theta"], dtype=np.float32)

    in_maps = _prep_inputs(x, att, cheb, Theta)
    nc = _build_bass()
    res = run_bass_kernel_spmd(nc, in_maps, core_ids=list(range(B)))
    _LAST_RESULTS = res

    out = np.stack(
        [r["out"].astype(np.float32).reshape(N, T, F_OUT).transpose(1, 0, 2)
         for r in res.results]
    )
    return out
